# revision 35
# baseline (speedup 1.0000x reference)
"""GQA attention kernel for Trainium2, 8 NeuronCores.

Problem: B=2, T=2048, D=1024, 16 Q heads / 4 KV heads, head_dim=64, RoPE,
causal softmax, out-projection.

Sharding: 8 cores = 2 (batch) x 4 (KV group). Core c handles batch c//4 and
KV group g=c%4 (query heads 4g..4g+3). wq/wk/wv column-sharded, wo
row-sharded; the 4 partial outputs per batch are summed on the host.

v2 architecture (vs the v1 baseline at 428us):
- Pipelined over 4 column chunks of T (512 q-positions each): projections,
  attention, and the output projection of the previous chunk interleave, so
  the PE never drains and the HBM loads/stores overlap compute.
- Softmax exp is split between the Scalar engine (native Exp activation) and
  the Vector engine (Schraudolph bit-trick exp: i32 = s*A + B, bitcast to
  f32 gives 2^(s*log2e*0.125) with ~3% sawtooth error that mostly cancels in
  the softmax normalization). One exp instruction covers a PAIR of score
  tiles ([128, 2, 512] PSUM) to amortize per-instruction overhead.
- Scores are computed transposed (scoresT[kpos, qpos]) so PV needs no
  transposes; the softmax denominator L rides along as a ones-column of V.
- 1/L is computed on a [128, 4] tile (DMA reshape of the one-partition L row)
  instead of a [1, 512] row: the Vector reciprocal is per-lane-serial, so the
  reshape makes it ~100x cheaper.
- Causal trimming: diagonal score tiles only compute/exp/PV the valid
  column range; the triangular boundary block gets a [128,128] mask add.
- RoPE multiplies run on the (otherwise idle) GpSimd engine; V is produced
  via PE transposes of the K/V projection; all PSUM<->SBUF copies are on
  Scalar, masks/normalize/reciprocal on Vector.

All matmuls are float32r (full fp32 data, fast PE mode).
"""

import numpy as np
import sys

sys.path.insert(0, "/opt/trn_rl_repo")

from concourse import bass, bacc, mybir, tile  # noqa: E402
from concourse.bass_utils import run_bass_kernel_spmd  # noqa: E402

F32 = mybir.dt.float32
F32R = mybir.dt.float32r
I16 = mybir.dt.int16
BF16 = mybir.dt.bfloat16

B, T, D = 2, 2048, 1024
HD = 64                      # head dim
NQH = 4                      # query heads per core
QCOLS = NQH * HD             # 256
KC = D // 128                # 8 contraction chunks
NCI = 4                      # 512-wide column chunks of T
NT = T // 128                # 16 k-position tiles
N_CORES = 8

LOG2E = 1.4426950408889634
EXPA = 0.125 * LOG2E * (1 << 7)           # fold the 1/sqrt(hd) scale in
EXPB = (127.0 - 0.05) * (1 << 7)          # Schraudolph bias, tuned offline
MASKV = -300.0

_cache = {}


def _r(ap):
    return ap.bitcast(F32R)


def build_nc():
    """Build the (SPMD-identical) single-core bass program."""
    nc = bacc.Bacc("TRN2", target_bir_lowering=False, debug=False)

    xT_d = nc.declare_dram_parameter("xT", [D, T], F32R, isOutput=False)
    wq_d = nc.declare_dram_parameter("wq", [D, QCOLS], F32R, isOutput=False)
    wkv_d = nc.declare_dram_parameter("wkv", [D, 128], F32R, isOutput=False)
    wo_d = nc.declare_dram_parameter("wo", [QCOLS, D], F32R, isOutput=False)
    cos_d = nc.declare_dram_parameter("cosf", [128, T], F32, isOutput=False)
    sin_d = nc.declare_dram_parameter("sinf", [128, T], F32, isOutput=False)
    msk_d = nc.declare_dram_parameter("msk", [128, 128], F32, isOutput=False)
    id_d = nc.declare_dram_parameter("ident", [128, HD], F32, isOutput=False)
    out_d = nc.declare_dram_parameter("out", [T, D], F32, isOutput=True)
    # DRAM scratch used to reshape 1/L rows ([128,4] -> [1,512]); the DMA
    # engine is the only path that can move data across SBUF partitions.
    scr_d = nc.declare_dram_parameter("scr", [16, 512], F32R, isOutput=True)
    ones_d = nc.declare_dram_parameter("ones1", [1, HD], F32R, isOutput=False)

    with tile.TileContext(nc) as tc:
        with (
            tc.tile_pool(name="sb", bufs=1) as sb,
            tc.tile_pool(name="atp", bufs=3) as atp,
            tc.tile_pool(name="aop", bufs=4) as aop,
            tc.tile_pool(name="aotp", bufs=2) as aotp,
            tc.tile_pool(name="otp", bufs=3) as otp,
            tc.tile_pool(name="rotp", bufs=2) as rotp,
            tc.tile_pool(name="vtp", bufs=2) as vtp,
            tc.tile_pool(name="lrp", bufs=2) as lrp,
            tc.tile_pool(name="scp", bufs=3, space="PSUM") as scp,
            tc.tile_pool(name="pvp", bufs=2, space="PSUM") as pvp,
        ):
            wq = sb.tile([128, KC, QCOLS], F32, tag="wq")
            wkv = sb.tile([128, KC, 128], F32, tag="wkv")
            wo = sb.tile([128, 2, D], F32, tag="wo")
            cosf = sb.tile([128, T], F32, tag="cosf")
            sinf = sb.tile([128, T], F32, tag="sinf")
            msk = sb.tile([128, 128], F32, tag="msk")
            ident = sb.tile([128, HD], F32, tag="ident")
            qT = [sb.tile([128, T], F32, tag=f"qT{hp}", name=f"qT{hp}")
                  for hp in range(2)]
            kT = sb.tile([128, T], F32, tag="kT")
            vs = sb.tile([128, NT, HD + 1], BF16, tag="vs")
            xc = [sb.tile([128, KC, 512], F32, tag=f"xc{ci}", name=f"xc{ci}")
                  for ci in range(NCI)]

            # --- input loads (order = DMA issue order on the sync queue) ---
            nc.sync.dma_start(
                _r(wq[:]), wq_d[:, :].rearrange("(a b) c -> b a c", a=KC))
            nc.sync.dma_start(
                _r(wkv[:]), wkv_d[:, :].rearrange("(a b) c -> b a c", a=KC))
            for k in range(KC):
                nc.sync.dma_start(
                    _r(xc[0][:, k, :]),
                    xT_d[k * 128:(k + 1) * 128, 0:512])
            nc.sync.dma_start(cosf[:], cos_d[:])
            nc.sync.dma_start(sinf[:], sin_d[:])
            nc.sync.dma_start(msk[:], msk_d[:])
            nc.sync.dma_start(ident[:], id_d[:])
            for k in range(KC):
                nc.sync.dma_start(
                    _r(xc[1][:, k, :]),
                    xT_d[k * 128:(k + 1) * 128, 512:1024])
            nc.sync.dma_start(
                _r(wo[:]), wo_d[:, :].rearrange("(a b) c -> b a c", a=2))
            for ci in range(2, NCI):
                nc.sync.dma_start(
                    _r(xc[ci][:]),
                    xT_d[:, ci * 512:(ci + 1) * 512].rearrange(
                        "(a b) c -> b a c", a=KC))

            ones1 = sb.tile([1, HD], F32, tag="ones1")
            nc.sync.dma_start(_r(ones1[:]), ones_d[:])
            nc.vector.memset(vs[:, :, HD:HD + 1], 1.0)

            def rope_chunk(t_ap, cs, nrows):
                """t = t*cos + rot_half(t)*sin on de-interleaved rows.

                rot DMAs swap 32-row halves of each 64 block; muls/adds run
                on GpSimd to keep Vector/Scalar free for softmax work.
                """
                rot = rotp.tile([128, 512], F32, tag="rot")
                for blk in range(nrows // 64):
                    r0 = blk * 64
                    nc.scalar.dma_start(rot[r0:r0 + 32, :],
                                        t_ap[r0 + 32:r0 + 64, cs])
                    nc.scalar.dma_start(rot[r0 + 32:r0 + 64, :],
                                        t_ap[r0:r0 + 32, cs])
                nc.vector.tensor_mul(_r(t_ap[0:nrows, cs]), t_ap[0:nrows, cs],
                                     cosf[0:nrows, cs])
                nc.vector.tensor_mul(rot[0:nrows, :], rot[0:nrows, :],
                                     sinf[0:nrows, cs])
                nc.vector.tensor_add(_r(t_ap[0:nrows, cs]), t_ap[0:nrows, cs],
                                     rot[0:nrows, :])

            def proj_q(ci, hp):
                cs = slice(ci * 512, (ci + 1) * 512)
                pq = scp.tile([128, 512], F32, tag="sc")
                for k in range(KC):
                    nc.tensor.matmul(
                        pq[:], _r(wq[:, k, hp * 128:(hp + 1) * 128]),
                        _r(xc[ci][:, k, :]),
                        start=(k == 0), stop=(k == KC - 1))
                nc.scalar.copy(_r(qT[hp][:, cs]), pq[:])
                rope_chunk(qT[hp], cs, 128)

            def proj_kv(ci):
                cs = slice(ci * 512, (ci + 1) * 512)
                pkv = scp.tile([128, 512], F32, tag="sc")
                for k in range(KC):
                    nc.tensor.matmul(
                        pkv[:], _r(wkv[:, k, :]), _r(xc[ci][:, k, :]),
                        start=(k == 0), stop=(k == KC - 1))
                nc.scalar.copy(_r(kT[0:64, cs]), pkv[0:64, :])
                vtmp = vtp.tile([128, 512], F32, tag="vtmp")
                nc.scalar.copy(vtmp[64:128, :], pkv[64:128, :])
                rope_chunk(kT, cs, 64)
                # duplicate roped kT into the upper partition half for the
                # odd-head score matmuls (engines can't cross partitions).
                nc.sync.dma_start(_r(kT[64:128, cs]), _r(kT[0:64, cs]))
                # transpose V chunks into natural [tpos, dim] orientation
                for tb in range(4):
                    pt = scp.tile([128, 512], F32, tag="sc")
                    nc.tensor.transpose(
                        pt[:, 0:HD], vtmp[64:128, tb * 128:(tb + 1) * 128],
                        ident[64:128, 0:HD])
                    nc.scalar.copy(vs[:, ci * 4 + tb, 0:HD], pt[:, 0:HD])

            def attn_head(h, ci, state):
                cs0 = ci * 512
                n_tj = 4 * (ci + 1)
                n_p = n_tj // 2
                hp, hr = divmod(h, 2)
                qrow = slice(hr * 64, hr * 64 + 64)
                pv = pvp.tile([HD + 1, 512], F32, tag="pv")

                def sc_emit(p):
                    """Score pair p: 2 matmuls -> exp -> boundary masks."""
                    sc = scp.tile([128, 2, 512], F32, tag="sc")
                    at = atp.tile([128, 2, 512], BF16, tag="at")
                    m0 = 2 * p - 4 * ci
                    wp = 128 * m0 if m0 > 0 else 0
                    for s in range(2):
                        tj = 2 * p + s
                        nc.tensor.matmul(
                            sc[:, s, wp:512],
                            _r(kT[qrow, tj * 128:(tj + 1) * 128]),
                            _r(qT[hp][qrow, cs0 + wp:cs0 + 512]),
                            start=True, stop=True)
                    if ci <= 1 or p % 2 == 0:
                        nc.scalar.activation(
                            at[:, :, wp:512], sc[:, :, wp:512],
                            mybir.ActivationFunctionType.Exp, scale=0.125)
                    else:
                        nc.vector.tensor_scalar(
                            at[:, :, wp:512].bitcast(I16), sc[:, :, wp:512],
                            EXPA, EXPB, mybir.AluOpType.mult,
                            mybir.AluOpType.add)
                    for s in range(2):
                        # causal boundary: zero above-diagonal entries of the
                        # triangular block (0/1 mask; at is SBUF so the
                        # otherwise-idle GpSimd engine can do it).
                        tj = 2 * p + s
                        m = tj - 4 * ci
                        if m >= 0:
                            nc.gpsimd.tensor_mul(
                                at[:, s, 128 * m:128 * m + 128],
                                at[:, s, 128 * m:128 * m + 128], msk[:])
                    return at

                def pv_emit(p, at):
                    for s in range(2):
                        tj = 2 * p + s
                        m = tj - 4 * ci
                        w0 = 128 * m if m > 0 else 0
                        nc.tensor.matmul(
                            pv[:, w0:512], vs[:, tj, :],
                            at[:, s, w0:512],
                            start=(tj == 0), stop=(tj == n_tj - 1),
                            skip_group_check=True)

                # Emit score pairs two ahead of their PV consumers so the
                # exp latency hides behind the next pairs' matmuls instead
                # of stalling the PE (needs 3 sc-pair PSUM slots).
                ats = [sc_emit(p) for p in range(min(2, n_p))]
                for p in range(n_p):
                    if p + 2 < n_p:
                        ats.append(sc_emit(p + 2))
                    pv_emit(p, ats[p])
                state[h] = (pv, hp, hr, None, None)

            def fin_a(h, ci, state):
                # Drain pv: copy L row + unnormalized ao to SBUF, transpose
                # the L row onto partitions ([128,4]) and take 1/L there
                # (engine ops are lane-serial on a 1-partition row, and DMA
                # is the only path that crosses partitions).
                pv, hp, hr, _, _ = state[h]
                lr = lrp.tile([HD + 1, 512], F32, tag="lr")
                nc.vector.tensor_copy(lr[HD:HD + 1, :], pv[HD:HD + 1, :])
                if hr == 0:
                    dst = ao_tiles[ci][hp][0:64, :]
                else:
                    tmp = aotp.tile([64, 512], F32, tag="aotmp", name="aotmp")
                    dst = tmp[:]
                nc.scalar.copy(_r(dst), pv[0:HD, :])  # frees pv for next head
                lcol = scp.tile([128, 512], F32, tag="sc", name="lcol")
                for c in range(4):
                    nc.tensor.transpose(
                        lcol[:, c:c + 1],
                        lr[HD:HD + 1, c * 128:(c + 1) * 128],
                        ident[HD:HD + 1, 0:1])
                linvr = lrp.tile([128, 4], F32, tag="linvr")
                with nc.allow_low_precision(reason="f32r 1/L"):
                    nc.vector.reciprocal(_r(linvr[:]), lcol[:, 0:4])
                state[h] = (pv, hp, hr, dst, linvr)

            def fin_b1(h, ci, state):
                # linvr[p, c] = 1/L[128c + p]: scatter back into qpos order.
                # Emitted one head late so the sync queue never waits on it.
                _, _, _, _, linvr = state[h]
                idx = 4 * ci + h
                nc.sync.dma_start(
                    scr_d[idx:idx + 1, :].rearrange(
                        "a (f p) -> (a p) f", p=128), _r(linvr[:]))

            def fin_b2(h, ci, state):
                # Gather 1/L as a [1,512] row, broadcast across 64 partitions
                # with a ones-stationary matmul, normalize ao in place.
                _, hp, hr, dst, _ = state[h]
                idx = 4 * ci + h
                linv = lrp.tile([1, 512], F32, tag="linv")
                nc.sync.dma_start(_r(linv[:]), scr_d[idx:idx + 1, :])
                lb = scp.tile([128, 512], F32, tag="sc", name="lb")
                nc.tensor.matmul(lb[0:64, :], _r(ones1[:]), _r(linv[:]),
                                 start=True, stop=True)
                nc.vector.tensor_mul(_r(dst), dst, lb[0:64, :])
                if hr == 1:
                    nc.sync.dma_start(_r(ao_tiles[ci][hp][64:128, :]),
                                      _r(dst))

            def attn(ci, ao_ci):
                state = {}
                for h in range(NQH):
                    attn_head(h, ci, state)
                    fin_a(h, ci, state)
                    if h >= 1:
                        fin_b1(h - 1, ci, state)
                    if h >= 2:
                        fin_b2(h - 2, ci, state)
                fin_b1(NQH - 1, ci, state)
                fin_b2(NQH - 2, ci, state)
                fin_b2(NQH - 1, ci, state)

            def outproj_tb(ci, tb):
                ao_ci = ao_tiles[ci]
                ta = (ci * 4 + tb) * 128
                ot = otp.tile([128, 2, 512], F32, tag="ot")
                for nh in range(2):
                    po = scp.tile([128, 512], F32, tag="sc")
                    for cc in range(2):
                        nc.tensor.matmul(
                            po[:],
                            _r(ao_ci[cc][:, tb * 128:(tb + 1) * 128]),
                            _r(wo[:, cc, nh * 512:(nh + 1) * 512]),
                            start=(cc == 0), stop=(cc == 1))
                    if nh == 0:
                        nc.scalar.copy(ot[:, nh, :], po[:])
                    else:
                        nc.vector.tensor_copy(ot[:, nh, :], po[:])
                nc.sync.dma_start(out_d[ta:ta + 128, :], ot[:])

            ao_tiles = {}
            for ci in range(NCI):
                ao_tiles[ci] = [
                    aop.tile([128, 512], F32, tag="ao", name=f"ao{ci}_{hp}")
                    for hp in range(2)]

            # Warm-up burst: a dependency-free accumulation run right after
            # the wq load trips the PE HAM clock-gate (4/8 -> 8/8, i.e.
            # 1.2 -> 2.4 GHz needs ~3.4us of sustained busy) before the real
            # work starts.
            warm = scp.tile([128, 512], F32, tag="sc", name="warm")
            for i in range(32):
                nc.tensor.matmul(warm[:, 0:QCOLS], _r(wq[:, 0, 0:128]),
                                 _r(wq[:, i % KC, :]),
                                 start=(i == 0), stop=(i == 31))

            # Emission order: projections run two chunks ahead of attention;
            # out-projection of chunk ci comes after proj(ci+2) so its ao
            # normalization has drained by the time the PE reaches it.
            for c0 in range(2):
                proj_q(c0, 0)
                proj_q(c0, 1)
                proj_kv(c0)
            for ci in range(NCI):
                attn(ci, ao_tiles[ci])
                if ci + 2 < NCI:
                    proj_q(ci + 2, 0)
                    proj_q(ci + 2, 1)
                    proj_kv(ci + 2)
                for tb in range(4):
                    outproj_tb(ci, tb)

    nc.compile()
    return nc


def _round_f32r(a):
    """Round fp32 to the fp32r grid (11-bit mantissa, round-to-nearest)."""
    bits = np.ascontiguousarray(a, np.float32).view(np.uint32)
    return ((bits + 0x800) & 0xFFFFF000).view(np.float32)


def make_in_maps(x, freqs_cos, freqs_sin, wq, wk, wv, wo):
    """Host-side sharding + layout prep. Returns per-core input dicts."""
    x = np.asarray(x, np.float32)
    fc = np.asarray(freqs_cos, np.float32)
    fs = np.asarray(freqs_sin, np.float32)
    wq = np.asarray(wq, np.float32)
    wk = np.asarray(wk, np.float32)
    wv = np.asarray(wv, np.float32)
    wo = np.asarray(wo, np.float32)

    perm = np.concatenate([np.arange(0, HD, 2), np.arange(1, HD, 2)])
    cosT = np.ascontiguousarray(fc.T)            # (32, T)
    sinT = np.ascontiguousarray(fs.T)
    cosf = np.concatenate([cosT] * 4, axis=0)    # (128, T)
    sinf = np.concatenate([-sinT, sinT, -sinT, sinT], axis=0)

    jj = np.arange(128)[:, None]
    ii = np.arange(128)[None, :]
    msk = np.where(jj <= ii, 1.0, 0.0).astype(np.float32)
    ident = np.tile(np.eye(HD, dtype=np.float32), (2, 1))

    in_maps = []
    for c in range(N_CORES):
        b, g = divmod(c, 4)
        wq_c = wq[:, g * QCOLS:(g + 1) * QCOLS]
        wq_c = np.ascontiguousarray(
            wq_c.reshape(D, NQH, HD)[:, :, perm].reshape(D, QCOLS))
        wk_c = wk[:, g * HD:(g + 1) * HD][:, perm]
        wv_c = wv[:, g * HD:(g + 1) * HD]
        wkv_c = np.ascontiguousarray(np.concatenate([wk_c, wv_c], axis=1))
        wo_c = np.ascontiguousarray(wo[g * QCOLS:(g + 1) * QCOLS, :])
        xT_c = np.ascontiguousarray(x[b].T)
        in_maps.append({
            "xT": _round_f32r(xT_c), "wq": _round_f32r(wq_c),
            "wkv": _round_f32r(wkv_c), "wo": _round_f32r(wo_c),
            "cosf": cosf, "sinf": sinf, "msk": msk, "ident": ident,
            "ones1": np.ones((1, HD), np.float32),
        })
    return in_maps


def run_on_cores(in_maps, trace=False, **kwargs):
    if "nc" not in _cache:
        _cache["nc"] = build_nc()
    return run_bass_kernel_spmd(
        _cache["nc"], in_maps, core_ids=list(range(N_CORES)), trace=trace,
        **kwargs)


def kernel(x, freqs_cos, freqs_sin, wq, wk, wv, wo):
    in_maps = make_in_maps(x, freqs_cos, freqs_sin, wq, wk, wv, wo)
    res = run_on_cores(in_maps)
    outs = [res.results[c]["out"] for c in range(N_CORES)]
    full = np.empty((B, T, D), np.float32)
    for b in range(B):
        full[b] = outs[4 * b] + outs[4 * b + 1] + outs[4 * b + 2] + outs[4 * b + 3]
    return full


# revision 38
# speedup vs baseline: 1.1752x; 1.1752x over previous
"""GQA attention kernel for Trainium2, 8 NeuronCores.

Problem: B=2, T=2048, D=1024, 16 Q heads / 4 KV heads, head_dim=64, RoPE,
causal softmax, out-projection.

Sharding: 8 cores = 2 (batch) x 4 (KV group). Core c handles batch c//4 and
KV group g=c%4 (query heads 4g..4g+3). wq/wk/wv column-sharded, wo
row-sharded; the 4 partial outputs per batch are summed on the host.

v2 architecture (vs the v1 baseline at 428us):
- Pipelined over 4 column chunks of T (512 q-positions each): projections,
  attention, and the output projection of the previous chunk interleave, so
  the PE never drains and the HBM loads/stores overlap compute.
- Softmax exp is split between the Scalar engine (native Exp activation) and
  the Vector engine (Schraudolph bit-trick exp: i32 = s*A + B, bitcast to
  f32 gives 2^(s*log2e*0.125) with ~3% sawtooth error that mostly cancels in
  the softmax normalization). One exp instruction covers a PAIR of score
  tiles ([128, 2, 512] PSUM) to amortize per-instruction overhead.
- Scores are computed transposed (scoresT[kpos, qpos]) so PV needs no
  transposes; the softmax denominator L rides along as a ones-column of V.
- 1/L is computed on a [128, 4] tile (DMA reshape of the one-partition L row)
  instead of a [1, 512] row: the Vector reciprocal is per-lane-serial, so the
  reshape makes it ~100x cheaper.
- Causal trimming: diagonal score tiles only compute/exp/PV the valid
  column range; the triangular boundary block gets a [128,128] mask add.
- RoPE multiplies run on the (otherwise idle) GpSimd engine; V is produced
  via PE transposes of the K/V projection; all PSUM<->SBUF copies are on
  Scalar, masks/normalize/reciprocal on Vector.

All matmuls are float32r (full fp32 data, fast PE mode).
"""

import numpy as np
import sys

sys.path.insert(0, "/opt/trn_rl_repo")

from concourse import bass, bacc, mybir, tile  # noqa: E402
from concourse.bass_utils import run_bass_kernel_spmd  # noqa: E402

F32 = mybir.dt.float32
F32R = mybir.dt.float32r
I16 = mybir.dt.int16
BF16 = mybir.dt.bfloat16

B, T, D = 2, 2048, 1024
HD = 64                      # head dim
NQH = 4                      # query heads per core
QCOLS = NQH * HD             # 256
KC = D // 128                # 8 contraction chunks
NCI = 4                      # 512-wide column chunks of T
NT = T // 128                # 16 k-position tiles
N_CORES = 8

LOG2E = 1.4426950408889634
EXPA = 0.125 * LOG2E * (1 << 7)           # fold the 1/sqrt(hd) scale in
EXPB = (127.0 - 0.05) * (1 << 7)          # Schraudolph bias, tuned offline
MASKV = -300.0

# Scheduling knobs (tuned via the cost-model timeline simulator)
CFG = {
    "rope_engine": "vector",   # "vector" | "gpsimd"
    "lookahead": 1,            # sc pairs emitted ahead of PV
    "pool_mode": "split",      # "shared3" | "split"
    "scp_bufs": 2, "pvp_bufs": 2, "pop_bufs": 2,
    "exp_all_scalar_ci": 1,    # ci <= this -> all exp on Scalar
    "warmup": 0,              # warm-up matmul count
}

_cache = {}


def _r(ap):
    return ap.bitcast(F32R)


def build_nc():
    """Build the (SPMD-identical) single-core bass program."""
    nc = bacc.Bacc("TRN2", target_bir_lowering=False, debug=False)

    xT_d = nc.declare_dram_parameter("xT", [D, T], F32R, isOutput=False)
    wq_d = nc.declare_dram_parameter("wq", [D, QCOLS], F32R, isOutput=False)
    wkv_d = nc.declare_dram_parameter("wkv", [D, 128], F32R, isOutput=False)
    wo_d = nc.declare_dram_parameter("wo", [QCOLS, D], F32R, isOutput=False)
    cos_d = nc.declare_dram_parameter("cosf", [128, T], F32, isOutput=False)
    sin_d = nc.declare_dram_parameter("sinf", [128, T], F32, isOutput=False)
    msk_d = nc.declare_dram_parameter("msk", [128, 128], F32, isOutput=False)
    id_d = nc.declare_dram_parameter("ident", [128, HD], F32, isOutput=False)
    out_d = nc.declare_dram_parameter("out", [T, D], F32, isOutput=True)
    # DRAM scratch used to reshape 1/L rows ([128,4] -> [1,512]); the DMA
    # engine is the only path that can move data across SBUF partitions.
    scr_d = nc.declare_dram_parameter("scr", [16, 512], F32R, isOutput=True)
    ones_d = nc.declare_dram_parameter("ones1", [1, HD], F32R, isOutput=False)

    with tile.TileContext(nc) as tc:
        with (
            tc.tile_pool(name="sb", bufs=1) as sb,
            tc.tile_pool(name="atp", bufs=3) as atp,
            tc.tile_pool(name="aop", bufs=4) as aop,
            tc.tile_pool(name="aotp", bufs=2) as aotp,
            tc.tile_pool(name="otp", bufs=3) as otp,
            tc.tile_pool(name="rotp", bufs=2) as rotp,
            tc.tile_pool(name="vtp", bufs=2) as vtp,
            tc.tile_pool(name="lrp", bufs=2) as lrp,
            tc.tile_pool(name="scp", bufs=CFG["scp_bufs"],
                         space="PSUM") as scp,
            tc.tile_pool(name="pvp", bufs=CFG["pvp_bufs"],
                         space="PSUM") as pvp,
            tc.tile_pool(name="pop", bufs=CFG["pop_bufs"],
                         space="PSUM") as pop,
        ):
            if CFG["pool_mode"] == "shared3":
                pop = scp  # single rotation serves everything but pv
            wq = sb.tile([128, KC, QCOLS], F32, tag="wq")
            wkv = sb.tile([128, KC, 128], F32, tag="wkv")
            wo = sb.tile([128, 2, D], F32, tag="wo")
            cosf = sb.tile([128, T], F32, tag="cosf")
            sinf = sb.tile([128, T], F32, tag="sinf")
            msk = sb.tile([128, 128], F32, tag="msk")
            ident = sb.tile([128, HD], F32, tag="ident")
            qT = [sb.tile([128, T], F32, tag=f"qT{hp}", name=f"qT{hp}")
                  for hp in range(2)]
            kT = sb.tile([128, T], F32, tag="kT")
            vs = sb.tile([128, NT, HD + 1], BF16, tag="vs")
            xc = [sb.tile([128, KC, 512], F32, tag=f"xc{ci}", name=f"xc{ci}")
                  for ci in range(NCI)]

            # --- input loads (order = DMA issue order on the sync queue) ---
            nc.sync.dma_start(
                _r(wq[:]), wq_d[:, :].rearrange("(a b) c -> b a c", a=KC))
            nc.sync.dma_start(
                _r(wkv[:]), wkv_d[:, :].rearrange("(a b) c -> b a c", a=KC))
            for k in range(KC):
                nc.sync.dma_start(
                    _r(xc[0][:, k, :]),
                    xT_d[k * 128:(k + 1) * 128, 0:512])
            nc.sync.dma_start(cosf[:], cos_d[:])
            nc.sync.dma_start(sinf[:], sin_d[:])
            nc.sync.dma_start(msk[:], msk_d[:])
            nc.sync.dma_start(ident[:], id_d[:])
            for k in range(KC):
                nc.sync.dma_start(
                    _r(xc[1][:, k, :]),
                    xT_d[k * 128:(k + 1) * 128, 512:1024])
            nc.sync.dma_start(
                _r(wo[:]), wo_d[:, :].rearrange("(a b) c -> b a c", a=2))
            for ci in range(2, NCI):
                nc.sync.dma_start(
                    _r(xc[ci][:]),
                    xT_d[:, ci * 512:(ci + 1) * 512].rearrange(
                        "(a b) c -> b a c", a=KC))

            ones1 = sb.tile([1, HD], F32, tag="ones1")
            nc.sync.dma_start(_r(ones1[:]), ones_d[:])
            nc.vector.memset(vs[:, :, HD:HD + 1], 1.0)

            def rope_chunk(t_ap, cs, nrows):
                """t = t*cos + rot_half(t)*sin on de-interleaved rows.

                rot DMAs swap 32-row halves of each 64 block; muls/adds run
                on GpSimd to keep Vector/Scalar free for softmax work.
                """
                rot = rotp.tile([128, 512], F32, tag="rot")
                for blk in range(nrows // 64):
                    r0 = blk * 64
                    nc.scalar.dma_start(rot[r0:r0 + 32, :],
                                        t_ap[r0 + 32:r0 + 64, cs])
                    nc.scalar.dma_start(rot[r0 + 32:r0 + 64, :],
                                        t_ap[r0:r0 + 32, cs])
                eng = nc.vector if CFG["rope_engine"] == "vector" else nc.gpsimd
                eng.tensor_mul(_r(t_ap[0:nrows, cs]), t_ap[0:nrows, cs],
                               cosf[0:nrows, cs])
                eng.tensor_mul(rot[0:nrows, :], rot[0:nrows, :],
                               sinf[0:nrows, cs])
                eng.tensor_add(_r(t_ap[0:nrows, cs]), t_ap[0:nrows, cs],
                               rot[0:nrows, :])

            def proj_q(ci, hp):
                cs = slice(ci * 512, (ci + 1) * 512)
                pq = pop.tile([128, 512], F32, tag="sc")
                for k in range(KC):
                    nc.tensor.matmul(
                        pq[:], _r(wq[:, k, hp * 128:(hp + 1) * 128]),
                        _r(xc[ci][:, k, :]),
                        start=(k == 0), stop=(k == KC - 1))
                nc.scalar.copy(_r(qT[hp][:, cs]), pq[:])
                rope_chunk(qT[hp], cs, 128)

            def proj_kv(ci):
                cs = slice(ci * 512, (ci + 1) * 512)
                pkv = pop.tile([128, 512], F32, tag="sc")
                for k in range(KC):
                    nc.tensor.matmul(
                        pkv[:], _r(wkv[:, k, :]), _r(xc[ci][:, k, :]),
                        start=(k == 0), stop=(k == KC - 1))
                nc.scalar.copy(_r(kT[0:64, cs]), pkv[0:64, :])
                vtmp = vtp.tile([128, 512], F32, tag="vtmp")
                nc.scalar.copy(vtmp[64:128, :], pkv[64:128, :])
                rope_chunk(kT, cs, 64)
                # duplicate roped kT into the upper partition half for the
                # odd-head score matmuls (engines can't cross partitions).
                nc.sync.dma_start(_r(kT[64:128, cs]), _r(kT[0:64, cs]))
                # transpose V chunks into natural [tpos, dim] orientation
                for tb in range(4):
                    pt = pop.tile([128, 512], F32, tag="sc")
                    nc.tensor.transpose(
                        pt[:, 0:HD], vtmp[64:128, tb * 128:(tb + 1) * 128],
                        ident[64:128, 0:HD])
                    nc.scalar.copy(vs[:, ci * 4 + tb, 0:HD], pt[:, 0:HD])

            def attn_head(h, ci, state):
                cs0 = ci * 512
                n_tj = 4 * (ci + 1)
                n_p = n_tj // 2
                hp, hr = divmod(h, 2)
                qrow = slice(hr * 64, hr * 64 + 64)
                pv = pvp.tile([HD + 1, 512], F32, tag="pv")

                def sc_emit(p):
                    """Score pair p: 2 matmuls -> exp -> boundary masks."""
                    sc = scp.tile([128, 2, 512], F32, tag="sc")
                    at = atp.tile([128, 2, 512], BF16, tag="at")
                    m0 = 2 * p - 4 * ci
                    wp = 128 * m0 if m0 > 0 else 0
                    for s in range(2):
                        tj = 2 * p + s
                        nc.tensor.matmul(
                            sc[:, s, wp:512],
                            _r(kT[qrow, tj * 128:(tj + 1) * 128]),
                            _r(qT[hp][qrow, cs0 + wp:cs0 + 512]),
                            start=True, stop=True)
                    if ci <= CFG["exp_all_scalar_ci"] or p % 2 == 0:
                        nc.scalar.activation(
                            at[:, :, wp:512], sc[:, :, wp:512],
                            mybir.ActivationFunctionType.Exp, scale=0.125)
                    else:
                        nc.vector.tensor_scalar(
                            at[:, :, wp:512].bitcast(I16), sc[:, :, wp:512],
                            EXPA, EXPB, mybir.AluOpType.mult,
                            mybir.AluOpType.add)
                    for s in range(2):
                        # causal boundary: zero above-diagonal entries of the
                        # triangular block (0/1 mask; at is SBUF so the
                        # otherwise-idle GpSimd engine can do it).
                        tj = 2 * p + s
                        m = tj - 4 * ci
                        if m >= 0:
                            nc.gpsimd.tensor_mul(
                                at[:, s, 128 * m:128 * m + 128],
                                at[:, s, 128 * m:128 * m + 128], msk[:])
                    return at

                def pv_emit(p, at):
                    for s in range(2):
                        tj = 2 * p + s
                        m = tj - 4 * ci
                        w0 = 128 * m if m > 0 else 0
                        nc.tensor.matmul(
                            pv[:, w0:512], vs[:, tj, :],
                            at[:, s, w0:512],
                            start=(tj == 0), stop=(tj == n_tj - 1),
                            skip_group_check=True)

                # Emit score pairs two ahead of their PV consumers so the
                # exp latency hides behind the next pairs' matmuls instead
                # of stalling the PE (needs 3 sc-pair PSUM slots).
                la = CFG["lookahead"]
                ats = [sc_emit(p) for p in range(min(la, n_p))]
                for p in range(n_p):
                    if p + la < n_p:
                        ats.append(sc_emit(p + la))
                    pv_emit(p, ats[p])
                state[h] = (pv, hp, hr, None, None)

            def fin_a(h, ci, state):
                # Drain pv: copy L row + unnormalized ao to SBUF, transpose
                # the L row onto partitions ([128,4]) and take 1/L there
                # (engine ops are lane-serial on a 1-partition row, and DMA
                # is the only path that crosses partitions).
                pv, hp, hr, _, _ = state[h]
                lr = lrp.tile([HD + 1, 512], F32, tag="lr")
                nc.vector.tensor_copy(lr[HD:HD + 1, :], pv[HD:HD + 1, :])
                if hr == 0:
                    dst = ao_tiles[ci][hp][0:64, :]
                else:
                    tmp = aotp.tile([64, 512], F32, tag="aotmp", name="aotmp")
                    dst = tmp[:]
                nc.scalar.copy(_r(dst), pv[0:HD, :])  # frees pv for next head
                lcol = pop.tile([128, 512], F32, tag="sc", name="lcol")
                for c in range(4):
                    nc.tensor.transpose(
                        lcol[:, c:c + 1],
                        lr[HD:HD + 1, c * 128:(c + 1) * 128],
                        ident[HD:HD + 1, 0:1])
                linvr = lrp.tile([128, 4], F32, tag="linvr")
                with nc.allow_low_precision(reason="f32r 1/L"):
                    nc.vector.reciprocal(_r(linvr[:]), lcol[:, 0:4])
                state[h] = (pv, hp, hr, dst, linvr)

            def fin_b1(h, ci, state):
                # linvr[p, c] = 1/L[128c + p]: scatter back into qpos order.
                # Emitted one head late so the sync queue never waits on it.
                _, _, _, _, linvr = state[h]
                idx = 4 * ci + h
                nc.sync.dma_start(
                    scr_d[idx:idx + 1, :].rearrange(
                        "a (f p) -> (a p) f", p=128), _r(linvr[:]))

            def fin_b2(h, ci, state):
                # Gather 1/L as a [1,512] row, broadcast across 64 partitions
                # with a ones-stationary matmul, normalize ao in place.
                _, hp, hr, dst, _ = state[h]
                idx = 4 * ci + h
                linv = lrp.tile([1, 512], F32, tag="linv")
                nc.sync.dma_start(_r(linv[:]), scr_d[idx:idx + 1, :])
                lb = pop.tile([128, 512], F32, tag="sc", name="lb")
                nc.tensor.matmul(lb[0:64, :], _r(ones1[:]), _r(linv[:]),
                                 start=True, stop=True)
                nc.vector.tensor_mul(_r(dst), dst, lb[0:64, :])
                if hr == 1:
                    nc.sync.dma_start(_r(ao_tiles[ci][hp][64:128, :]),
                                      _r(dst))

            def attn(ci, ao_ci):
                state = {}
                for h in range(NQH):
                    attn_head(h, ci, state)
                    fin_a(h, ci, state)
                    if h >= 1:
                        fin_b1(h - 1, ci, state)
                    if h >= 2:
                        fin_b2(h - 2, ci, state)
                fin_b1(NQH - 1, ci, state)
                fin_b2(NQH - 2, ci, state)
                fin_b2(NQH - 1, ci, state)

            def outproj_tb(ci, tb):
                ao_ci = ao_tiles[ci]
                ta = (ci * 4 + tb) * 128
                ot = otp.tile([128, 2, 512], F32, tag="ot")
                for nh in range(2):
                    po = pop.tile([128, 512], F32, tag="sc")
                    for cc in range(2):
                        nc.tensor.matmul(
                            po[:],
                            _r(ao_ci[cc][:, tb * 128:(tb + 1) * 128]),
                            _r(wo[:, cc, nh * 512:(nh + 1) * 512]),
                            start=(cc == 0), stop=(cc == 1))
                    if nh == 0:
                        nc.scalar.copy(ot[:, nh, :], po[:])
                    else:
                        nc.vector.tensor_copy(ot[:, nh, :], po[:])
                nc.sync.dma_start(out_d[ta:ta + 128, :], ot[:])

            ao_tiles = {}
            for ci in range(NCI):
                ao_tiles[ci] = [
                    aop.tile([128, 512], F32, tag="ao", name=f"ao{ci}_{hp}")
                    for hp in range(2)]

            # Warm-up burst: a dependency-free accumulation run right after
            # the wq load trips the PE HAM clock-gate (4/8 -> 8/8, i.e.
            # 1.2 -> 2.4 GHz needs ~3.4us of sustained busy) before the real
            # work starts.
            nwarm = CFG["warmup"]
            if nwarm:
                warm = pop.tile([128, 512], F32, tag="sc", name="warm")
                for i in range(nwarm):
                    nc.tensor.matmul(warm[:, 0:QCOLS], _r(wq[:, 0, 0:128]),
                                     _r(wq[:, i % KC, :]),
                                     start=(i == 0), stop=(i == nwarm - 1))

            # Emission order: projections run two chunks ahead of attention;
            # out-projection of chunk ci comes after proj(ci+2) so its ao
            # normalization has drained by the time the PE reaches it.
            for c0 in range(2):
                proj_q(c0, 0)
                proj_q(c0, 1)
                proj_kv(c0)
            for ci in range(NCI):
                attn(ci, ao_tiles[ci])
                if ci + 2 < NCI:
                    proj_q(ci + 2, 0)
                    proj_q(ci + 2, 1)
                    proj_kv(ci + 2)
                for tb in range(4):
                    outproj_tb(ci, tb)

    nc.compile()
    return nc


def _round_f32r(a):
    """Round fp32 to the fp32r grid (11-bit mantissa, round-to-nearest)."""
    bits = np.ascontiguousarray(a, np.float32).view(np.uint32)
    return ((bits + 0x800) & 0xFFFFF000).view(np.float32)


def make_in_maps(x, freqs_cos, freqs_sin, wq, wk, wv, wo):
    """Host-side sharding + layout prep. Returns per-core input dicts."""
    x = np.asarray(x, np.float32)
    fc = np.asarray(freqs_cos, np.float32)
    fs = np.asarray(freqs_sin, np.float32)
    wq = np.asarray(wq, np.float32)
    wk = np.asarray(wk, np.float32)
    wv = np.asarray(wv, np.float32)
    wo = np.asarray(wo, np.float32)

    perm = np.concatenate([np.arange(0, HD, 2), np.arange(1, HD, 2)])
    cosT = np.ascontiguousarray(fc.T)            # (32, T)
    sinT = np.ascontiguousarray(fs.T)
    cosf = np.concatenate([cosT] * 4, axis=0)    # (128, T)
    sinf = np.concatenate([-sinT, sinT, -sinT, sinT], axis=0)

    jj = np.arange(128)[:, None]
    ii = np.arange(128)[None, :]
    msk = np.where(jj <= ii, 1.0, 0.0).astype(np.float32)
    ident = np.tile(np.eye(HD, dtype=np.float32), (2, 1))

    in_maps = []
    for c in range(N_CORES):
        b, g = divmod(c, 4)
        wq_c = wq[:, g * QCOLS:(g + 1) * QCOLS]
        wq_c = np.ascontiguousarray(
            wq_c.reshape(D, NQH, HD)[:, :, perm].reshape(D, QCOLS))
        wk_c = wk[:, g * HD:(g + 1) * HD][:, perm]
        wv_c = wv[:, g * HD:(g + 1) * HD]
        wkv_c = np.ascontiguousarray(np.concatenate([wk_c, wv_c], axis=1))
        wo_c = np.ascontiguousarray(wo[g * QCOLS:(g + 1) * QCOLS, :])
        xT_c = np.ascontiguousarray(x[b].T)
        in_maps.append({
            "xT": _round_f32r(xT_c), "wq": _round_f32r(wq_c),
            "wkv": _round_f32r(wkv_c), "wo": _round_f32r(wo_c),
            "cosf": cosf, "sinf": sinf, "msk": msk, "ident": ident,
            "ones1": np.ones((1, HD), np.float32),
        })
    return in_maps


def run_on_cores(in_maps, trace=False, **kwargs):
    if "nc" not in _cache:
        _cache["nc"] = build_nc()
    return run_bass_kernel_spmd(
        _cache["nc"], in_maps, core_ids=list(range(N_CORES)), trace=trace,
        **kwargs)


def kernel(x, freqs_cos, freqs_sin, wq, wk, wv, wo):
    in_maps = make_in_maps(x, freqs_cos, freqs_sin, wq, wk, wv, wo)
    res = run_on_cores(in_maps)
    outs = [res.results[c]["out"] for c in range(N_CORES)]
    full = np.empty((B, T, D), np.float32)
    for b in range(B):
        full[b] = outs[4 * b] + outs[4 * b + 1] + outs[4 * b + 2] + outs[4 * b + 3]
    return full


# revision 41
# speedup vs baseline: 1.1887x; 1.0115x over previous
"""GQA attention kernel for Trainium2, 8 NeuronCores.

Problem: B=2, T=2048, D=1024, 16 Q heads / 4 KV heads, head_dim=64, RoPE,
causal softmax, out-projection.

Sharding: 8 cores = 2 (batch) x 4 (KV group). Core c handles batch c//4 and
KV group g=c%4 (query heads 4g..4g+3). wq/wk/wv column-sharded, wo
row-sharded; the 4 partial outputs per batch are summed on the host.

v2 architecture (vs the v1 baseline at 428us):
- Pipelined over 4 column chunks of T (512 q-positions each): projections,
  attention, and the output projection of the previous chunk interleave, so
  the PE never drains and the HBM loads/stores overlap compute.
- Softmax exp is split between the Scalar engine (native Exp activation) and
  the Vector engine (Schraudolph bit-trick exp: i32 = s*A + B, bitcast to
  f32 gives 2^(s*log2e*0.125) with ~3% sawtooth error that mostly cancels in
  the softmax normalization). One exp instruction covers a PAIR of score
  tiles ([128, 2, 512] PSUM) to amortize per-instruction overhead.
- Scores are computed transposed (scoresT[kpos, qpos]) so PV needs no
  transposes; the softmax denominator L rides along as a ones-column of V.
- 1/L is computed on a [128, 4] tile (DMA reshape of the one-partition L row)
  instead of a [1, 512] row: the Vector reciprocal is per-lane-serial, so the
  reshape makes it ~100x cheaper.
- Causal trimming: diagonal score tiles only compute/exp/PV the valid
  column range; the triangular boundary block gets a [128,128] mask add.
- RoPE multiplies run on the (otherwise idle) GpSimd engine; V is produced
  via PE transposes of the K/V projection; all PSUM<->SBUF copies are on
  Scalar, masks/normalize/reciprocal on Vector.

All matmuls are float32r (full fp32 data, fast PE mode).
"""

import numpy as np
import sys

sys.path.insert(0, "/opt/trn_rl_repo")

from concourse import bass, bacc, mybir, tile  # noqa: E402
from concourse.bass_utils import run_bass_kernel_spmd  # noqa: E402

F32 = mybir.dt.float32
F32R = mybir.dt.float32r
I16 = mybir.dt.int16
BF16 = mybir.dt.bfloat16

B, T, D = 2, 2048, 1024
HD = 64                      # head dim
NQH = 4                      # query heads per core
QCOLS = NQH * HD             # 256
KC = D // 128                # 8 contraction chunks
NCI = 4                      # 512-wide column chunks of T
NT = T // 128                # 16 k-position tiles
N_CORES = 8

LOG2E = 1.4426950408889634
EXPA = 0.125 * LOG2E * (1 << 7)           # fold the 1/sqrt(hd) scale in
EXPB = (127.0 - 0.05) * (1 << 7)          # Schraudolph bias, tuned offline
MASKV = -300.0

# Scheduling knobs (tuned via the cost-model timeline simulator)
CFG = {
    "rope_engine": "vector",   # "vector" | "gpsimd"
    "lookahead": 1,            # sc pairs emitted ahead of PV
    "pool_mode": "split",      # "shared3" | "split"
    "scp_bufs": 2, "pvp_bufs": 2, "pop_bufs": 2,
    "exp_all_scalar_ci": 1,    # ci <= this -> all exp on Scalar
    "warmup": 32,              # warm-up matmul count
    "op_interleave": True,     # emit outproj tiles between attention heads
    "keepwarm": 0,             # dummy matmuls per head in late chunks
}

_cache = {}


def _r(ap):
    return ap.bitcast(F32R)


def build_nc():
    """Build the (SPMD-identical) single-core bass program."""
    nc = bacc.Bacc("TRN2", target_bir_lowering=False, debug=False)

    xT_d = nc.declare_dram_parameter("xT", [D, T], F32R, isOutput=False)
    wq_d = nc.declare_dram_parameter("wq", [D, QCOLS], F32R, isOutput=False)
    wkv_d = nc.declare_dram_parameter("wkv", [D, 128], F32R, isOutput=False)
    wo_d = nc.declare_dram_parameter("wo", [QCOLS, D], F32R, isOutput=False)
    cos_d = nc.declare_dram_parameter("cosf", [128, T], F32, isOutput=False)
    sin_d = nc.declare_dram_parameter("sinf", [128, T], F32, isOutput=False)
    msk_d = nc.declare_dram_parameter("msk", [128, 128], F32, isOutput=False)
    id_d = nc.declare_dram_parameter("ident", [128, HD], F32, isOutput=False)
    out_d = nc.declare_dram_parameter("out", [T, D], F32, isOutput=True)
    # DRAM scratch used to reshape 1/L rows ([128,4] -> [1,512]); the DMA
    # engine is the only path that can move data across SBUF partitions.
    scr_d = nc.declare_dram_parameter("scr", [16, 512], F32R, isOutput=True)
    ones_d = nc.declare_dram_parameter("ones1", [1, HD], F32R, isOutput=False)

    with tile.TileContext(nc) as tc:
        with (
            tc.tile_pool(name="sb", bufs=1) as sb,
            tc.tile_pool(name="atp", bufs=3) as atp,
            tc.tile_pool(name="aop", bufs=4) as aop,
            tc.tile_pool(name="aotp", bufs=2) as aotp,
            tc.tile_pool(name="otp", bufs=3) as otp,
            tc.tile_pool(name="rotp", bufs=2) as rotp,
            tc.tile_pool(name="vtp", bufs=2) as vtp,
            tc.tile_pool(name="lrp", bufs=2) as lrp,
            tc.tile_pool(name="scp", bufs=CFG["scp_bufs"],
                         space="PSUM") as scp,
            tc.tile_pool(name="pvp", bufs=CFG["pvp_bufs"],
                         space="PSUM") as pvp,
            tc.tile_pool(name="pop", bufs=CFG["pop_bufs"],
                         space="PSUM") as pop,
        ):
            if CFG["pool_mode"] == "shared3":
                pop = scp  # single rotation serves everything but pv
            wq = sb.tile([128, KC, QCOLS], F32, tag="wq")
            wkv = sb.tile([128, KC, 128], F32, tag="wkv")
            wo = sb.tile([128, 2, D], F32, tag="wo")
            cosf = sb.tile([128, T], F32, tag="cosf")
            sinf = sb.tile([128, T], F32, tag="sinf")
            msk = sb.tile([128, 128], F32, tag="msk")
            ident = sb.tile([128, HD], F32, tag="ident")
            qT = [sb.tile([128, T], F32, tag=f"qT{hp}", name=f"qT{hp}")
                  for hp in range(2)]
            kT = sb.tile([128, T], F32, tag="kT")
            vs = sb.tile([128, NT, HD + 1], BF16, tag="vs")
            xc = [sb.tile([128, KC, 512], F32, tag=f"xc{ci}", name=f"xc{ci}")
                  for ci in range(NCI)]

            # --- input loads (order = DMA issue order on the sync queue) ---
            nc.sync.dma_start(
                _r(wq[:]), wq_d[:, :].rearrange("(a b) c -> b a c", a=KC))
            nc.sync.dma_start(
                _r(wkv[:]), wkv_d[:, :].rearrange("(a b) c -> b a c", a=KC))
            for k in range(KC):
                nc.sync.dma_start(
                    _r(xc[0][:, k, :]),
                    xT_d[k * 128:(k + 1) * 128, 0:512])
            nc.sync.dma_start(cosf[:], cos_d[:])
            nc.sync.dma_start(sinf[:], sin_d[:])
            nc.sync.dma_start(msk[:], msk_d[:])
            nc.sync.dma_start(ident[:], id_d[:])
            for k in range(KC):
                nc.sync.dma_start(
                    _r(xc[1][:, k, :]),
                    xT_d[k * 128:(k + 1) * 128, 512:1024])
            nc.sync.dma_start(
                _r(wo[:]), wo_d[:, :].rearrange("(a b) c -> b a c", a=2))
            for ci in range(2, NCI):
                nc.sync.dma_start(
                    _r(xc[ci][:]),
                    xT_d[:, ci * 512:(ci + 1) * 512].rearrange(
                        "(a b) c -> b a c", a=KC))

            ones1 = sb.tile([1, HD], F32, tag="ones1")
            nc.sync.dma_start(_r(ones1[:]), ones_d[:])
            nc.vector.memset(vs[:, :, HD:HD + 1], 1.0)

            def rope_chunk(t_ap, cs, nrows):
                """t = t*cos + rot_half(t)*sin on de-interleaved rows.

                rot DMAs swap 32-row halves of each 64 block; muls/adds run
                on GpSimd to keep Vector/Scalar free for softmax work.
                """
                rot = rotp.tile([128, 512], F32, tag="rot")
                for blk in range(nrows // 64):
                    r0 = blk * 64
                    nc.scalar.dma_start(rot[r0:r0 + 32, :],
                                        t_ap[r0 + 32:r0 + 64, cs])
                    nc.scalar.dma_start(rot[r0 + 32:r0 + 64, :],
                                        t_ap[r0:r0 + 32, cs])
                eng = nc.vector if CFG["rope_engine"] == "vector" else nc.gpsimd
                eng.tensor_mul(_r(t_ap[0:nrows, cs]), t_ap[0:nrows, cs],
                               cosf[0:nrows, cs])
                eng.tensor_mul(rot[0:nrows, :], rot[0:nrows, :],
                               sinf[0:nrows, cs])
                eng.tensor_add(_r(t_ap[0:nrows, cs]), t_ap[0:nrows, cs],
                               rot[0:nrows, :])

            def proj_q(ci, hp):
                cs = slice(ci * 512, (ci + 1) * 512)
                pq = pop.tile([128, 512], F32, tag="sc")
                for k in range(KC):
                    nc.tensor.matmul(
                        pq[:], _r(wq[:, k, hp * 128:(hp + 1) * 128]),
                        _r(xc[ci][:, k, :]),
                        start=(k == 0), stop=(k == KC - 1))
                nc.scalar.copy(_r(qT[hp][:, cs]), pq[:])
                rope_chunk(qT[hp], cs, 128)

            def proj_kv(ci):
                cs = slice(ci * 512, (ci + 1) * 512)
                pkv = pop.tile([128, 512], F32, tag="sc")
                for k in range(KC):
                    nc.tensor.matmul(
                        pkv[:], _r(wkv[:, k, :]), _r(xc[ci][:, k, :]),
                        start=(k == 0), stop=(k == KC - 1))
                nc.scalar.copy(_r(kT[0:64, cs]), pkv[0:64, :])
                vtmp = vtp.tile([128, 512], F32, tag="vtmp")
                nc.scalar.copy(vtmp[64:128, :], pkv[64:128, :])
                rope_chunk(kT, cs, 64)
                # duplicate roped kT into the upper partition half for the
                # odd-head score matmuls (engines can't cross partitions).
                nc.sync.dma_start(_r(kT[64:128, cs]), _r(kT[0:64, cs]))
                # transpose V chunks into natural [tpos, dim] orientation
                for tb in range(4):
                    pt = pop.tile([128, 512], F32, tag="sc")
                    nc.tensor.transpose(
                        pt[:, 0:HD], vtmp[64:128, tb * 128:(tb + 1) * 128],
                        ident[64:128, 0:HD])
                    nc.scalar.copy(vs[:, ci * 4 + tb, 0:HD], pt[:, 0:HD])

            def attn_head(h, ci, state):
                cs0 = ci * 512
                n_tj = 4 * (ci + 1)
                n_p = n_tj // 2
                hp, hr = divmod(h, 2)
                qrow = slice(hr * 64, hr * 64 + 64)
                pv = pvp.tile([HD + 1, 512], F32, tag="pv")

                def sc_emit(p):
                    """Score pair p: 2 matmuls -> exp -> boundary masks."""
                    sc = scp.tile([128, 2, 512], F32, tag="sc")
                    at = atp.tile([128, 2, 512], BF16, tag="at")
                    m0 = 2 * p - 4 * ci
                    wp = 128 * m0 if m0 > 0 else 0
                    for s in range(2):
                        tj = 2 * p + s
                        nc.tensor.matmul(
                            sc[:, s, wp:512],
                            _r(kT[qrow, tj * 128:(tj + 1) * 128]),
                            _r(qT[hp][qrow, cs0 + wp:cs0 + 512]),
                            start=True, stop=True)
                    if ci <= CFG["exp_all_scalar_ci"] or p % 2 == 0:
                        nc.scalar.activation(
                            at[:, :, wp:512], sc[:, :, wp:512],
                            mybir.ActivationFunctionType.Exp, scale=0.125)
                    else:
                        nc.vector.tensor_scalar(
                            at[:, :, wp:512].bitcast(I16), sc[:, :, wp:512],
                            EXPA, EXPB, mybir.AluOpType.mult,
                            mybir.AluOpType.add)
                    for s in range(2):
                        # causal boundary: zero above-diagonal entries of the
                        # triangular block (0/1 mask; at is SBUF so the
                        # otherwise-idle GpSimd engine can do it).
                        tj = 2 * p + s
                        m = tj - 4 * ci
                        if m >= 0:
                            nc.gpsimd.tensor_mul(
                                at[:, s, 128 * m:128 * m + 128],
                                at[:, s, 128 * m:128 * m + 128], msk[:])
                    return at

                def pv_emit(p, at):
                    for s in range(2):
                        tj = 2 * p + s
                        m = tj - 4 * ci
                        w0 = 128 * m if m > 0 else 0
                        nc.tensor.matmul(
                            pv[:, w0:512], vs[:, tj, :],
                            at[:, s, w0:512],
                            start=(tj == 0), stop=(tj == n_tj - 1),
                            skip_group_check=True)

                # Emit score pairs two ahead of their PV consumers so the
                # exp latency hides behind the next pairs' matmuls instead
                # of stalling the PE (needs 3 sc-pair PSUM slots).
                la = CFG["lookahead"]
                ats = [sc_emit(p) for p in range(min(la, n_p))]
                for p in range(n_p):
                    if p + la < n_p:
                        ats.append(sc_emit(p + la))
                    pv_emit(p, ats[p])
                state[h] = (pv, hp, hr, None, None)

            def fin_a(h, ci, state):
                # Drain pv: copy L row + unnormalized ao to SBUF, transpose
                # the L row onto partitions ([128,4]) and take 1/L there
                # (engine ops are lane-serial on a 1-partition row, and DMA
                # is the only path that crosses partitions).
                pv, hp, hr, _, _ = state[h]
                lr = lrp.tile([HD + 1, 512], F32, tag="lr")
                nc.vector.tensor_copy(lr[HD:HD + 1, :], pv[HD:HD + 1, :])
                if hr == 0:
                    dst = ao_tiles[ci][hp][0:64, :]
                else:
                    tmp = aotp.tile([64, 512], F32, tag="aotmp", name="aotmp")
                    dst = tmp[:]
                nc.scalar.copy(_r(dst), pv[0:HD, :])  # frees pv for next head
                lcol = pop.tile([128, 512], F32, tag="sc", name="lcol")
                for c in range(4):
                    nc.tensor.transpose(
                        lcol[:, c:c + 1],
                        lr[HD:HD + 1, c * 128:(c + 1) * 128],
                        ident[HD:HD + 1, 0:1])
                linvr = lrp.tile([128, 4], F32, tag="linvr")
                with nc.allow_low_precision(reason="f32r 1/L"):
                    nc.vector.reciprocal(_r(linvr[:]), lcol[:, 0:4])
                state[h] = (pv, hp, hr, dst, linvr)

            def fin_b1(h, ci, state):
                # linvr[p, c] = 1/L[128c + p]: scatter back into qpos order.
                # Emitted one head late so the sync queue never waits on it.
                _, _, _, _, linvr = state[h]
                idx = 4 * ci + h
                nc.sync.dma_start(
                    scr_d[idx:idx + 1, :].rearrange(
                        "a (f p) -> (a p) f", p=128), _r(linvr[:]))

            def fin_b2(h, ci, state):
                # Gather 1/L as a [1,512] row, broadcast across 64 partitions
                # with a ones-stationary matmul, normalize ao in place.
                _, hp, hr, dst, _ = state[h]
                idx = 4 * ci + h
                linv = lrp.tile([1, 512], F32, tag="linv")
                nc.sync.dma_start(_r(linv[:]), scr_d[idx:idx + 1, :])
                lb = pop.tile([128, 512], F32, tag="sc", name="lb")
                nc.tensor.matmul(lb[0:64, :], _r(ones1[:]), _r(linv[:]),
                                 start=True, stop=True)
                nc.vector.tensor_mul(_r(dst), dst, lb[0:64, :])
                if hr == 1:
                    nc.sync.dma_start(_r(ao_tiles[ci][hp][64:128, :]),
                                      _r(dst))

            def keepwarm_burst(n):
                wt = pop.tile([128, 512], F32, tag="sc", name="kw")
                for i in range(n):
                    nc.tensor.matmul(wt[:, 0:QCOLS], _r(wq[:, 0, 0:128]),
                                     _r(wq[:, i % KC, :]),
                                     start=(i == 0), stop=(i == n - 1))

            def attn(ci, ao_ci, parts=()):
                parts = list(parts)
                state = {}
                for h in range(NQH):
                    attn_head(h, ci, state)
                    fin_a(h, ci, state)
                    if h >= 1:
                        fin_b1(h - 1, ci, state)
                    if h >= 2:
                        fin_b2(h - 2, ci, state)
                    if parts:
                        parts.pop(0)()
                    if CFG["keepwarm"] and ci >= 2:
                        keepwarm_burst(CFG["keepwarm"])
                fin_b1(NQH - 1, ci, state)
                fin_b2(NQH - 2, ci, state)
                fin_b2(NQH - 1, ci, state)
                for p in parts:
                    p()

            def outproj_tb(ci, tb):
                ao_ci = ao_tiles[ci]
                ta = (ci * 4 + tb) * 128
                ot = otp.tile([128, 2, 512], F32, tag="ot")
                for nh in range(2):
                    po = pop.tile([128, 512], F32, tag="sc")
                    for cc in range(2):
                        nc.tensor.matmul(
                            po[:],
                            _r(ao_ci[cc][:, tb * 128:(tb + 1) * 128]),
                            _r(wo[:, cc, nh * 512:(nh + 1) * 512]),
                            start=(cc == 0), stop=(cc == 1))
                    if nh == 0:
                        nc.scalar.copy(ot[:, nh, :], po[:])
                    else:
                        nc.vector.tensor_copy(ot[:, nh, :], po[:])
                nc.sync.dma_start(out_d[ta:ta + 128, :], ot[:])

            ao_tiles = {}
            for ci in range(NCI):
                ao_tiles[ci] = [
                    aop.tile([128, 512], F32, tag="ao", name=f"ao{ci}_{hp}")
                    for hp in range(2)]

            # Warm-up burst: a dependency-free accumulation run right after
            # the wq load trips the PE HAM clock-gate (4/8 -> 8/8, i.e.
            # 1.2 -> 2.4 GHz needs ~3.4us of sustained busy) before the real
            # work starts.
            nwarm = CFG["warmup"]
            if nwarm:
                warm = pop.tile([128, 512], F32, tag="sc", name="warm")
                for i in range(nwarm):
                    nc.tensor.matmul(warm[:, 0:QCOLS], _r(wq[:, 0, 0:128]),
                                     _r(wq[:, i % KC, :]),
                                     start=(i == 0), stop=(i == nwarm - 1))

            # Emission order: projections run two chunks ahead of attention;
            # out-projection of chunk ci comes after proj(ci+2) so its ao
            # normalization has drained by the time the PE reaches it.
            for c0 in range(2):
                proj_q(c0, 0)
                proj_q(c0, 1)
                proj_kv(c0)
            for ci in range(NCI):
                parts = []
                if CFG["op_interleave"] and ci >= 1:
                    parts = [lambda c=ci - 1, t=tb: outproj_tb(c, t)
                             for tb in range(4)]
                attn(ci, ao_tiles[ci], parts)
                if ci + 2 < NCI:
                    proj_q(ci + 2, 0)
                    proj_q(ci + 2, 1)
                    proj_kv(ci + 2)
                if not CFG["op_interleave"]:
                    for tb in range(4):
                        outproj_tb(ci, tb)
            if CFG["op_interleave"]:
                for tb in range(4):
                    outproj_tb(NCI - 1, tb)

    nc.compile()
    return nc


def _round_f32r(a):
    """Round fp32 to the fp32r grid (11-bit mantissa, round-to-nearest)."""
    bits = np.ascontiguousarray(a, np.float32).view(np.uint32)
    return ((bits + 0x800) & 0xFFFFF000).view(np.float32)


def make_in_maps(x, freqs_cos, freqs_sin, wq, wk, wv, wo):
    """Host-side sharding + layout prep. Returns per-core input dicts."""
    x = np.asarray(x, np.float32)
    fc = np.asarray(freqs_cos, np.float32)
    fs = np.asarray(freqs_sin, np.float32)
    wq = np.asarray(wq, np.float32)
    wk = np.asarray(wk, np.float32)
    wv = np.asarray(wv, np.float32)
    wo = np.asarray(wo, np.float32)

    perm = np.concatenate([np.arange(0, HD, 2), np.arange(1, HD, 2)])
    cosT = np.ascontiguousarray(fc.T)            # (32, T)
    sinT = np.ascontiguousarray(fs.T)
    cosf = np.concatenate([cosT] * 4, axis=0)    # (128, T)
    sinf = np.concatenate([-sinT, sinT, -sinT, sinT], axis=0)

    jj = np.arange(128)[:, None]
    ii = np.arange(128)[None, :]
    msk = np.where(jj <= ii, 1.0, 0.0).astype(np.float32)
    ident = np.tile(np.eye(HD, dtype=np.float32), (2, 1))

    in_maps = []
    for c in range(N_CORES):
        b, g = divmod(c, 4)
        wq_c = wq[:, g * QCOLS:(g + 1) * QCOLS]
        wq_c = np.ascontiguousarray(
            wq_c.reshape(D, NQH, HD)[:, :, perm].reshape(D, QCOLS))
        wk_c = wk[:, g * HD:(g + 1) * HD][:, perm]
        wv_c = wv[:, g * HD:(g + 1) * HD]
        wkv_c = np.ascontiguousarray(np.concatenate([wk_c, wv_c], axis=1))
        wo_c = np.ascontiguousarray(wo[g * QCOLS:(g + 1) * QCOLS, :])
        xT_c = np.ascontiguousarray(x[b].T)
        in_maps.append({
            "xT": _round_f32r(xT_c), "wq": _round_f32r(wq_c),
            "wkv": _round_f32r(wkv_c), "wo": _round_f32r(wo_c),
            "cosf": cosf, "sinf": sinf, "msk": msk, "ident": ident,
            "ones1": np.ones((1, HD), np.float32),
        })
    return in_maps


def run_on_cores(in_maps, trace=False, **kwargs):
    if "nc" not in _cache:
        _cache["nc"] = build_nc()
    return run_bass_kernel_spmd(
        _cache["nc"], in_maps, core_ids=list(range(N_CORES)), trace=trace,
        **kwargs)


def kernel(x, freqs_cos, freqs_sin, wq, wk, wv, wo):
    in_maps = make_in_maps(x, freqs_cos, freqs_sin, wq, wk, wv, wo)
    res = run_on_cores(in_maps)
    outs = [res.results[c]["out"] for c in range(N_CORES)]
    full = np.empty((B, T, D), np.float32)
    for b in range(B):
        full[b] = outs[4 * b] + outs[4 * b + 1] + outs[4 * b + 2] + outs[4 * b + 3]
    return full


# revision 44
# speedup vs baseline: 1.2307x; 1.0354x over previous
"""GQA attention kernel for Trainium2, 8 NeuronCores.

Problem: B=2, T=2048, D=1024, 16 Q heads / 4 KV heads, head_dim=64, RoPE,
causal softmax, out-projection.

Sharding: 8 cores = 2 (batch) x 4 (KV group). Core c handles batch c//4 and
KV group g=c%4 (query heads 4g..4g+3). wq/wk/wv column-sharded, wo
row-sharded; the 4 partial outputs per batch are summed on the host.

v2 architecture (vs the v1 baseline at 428us):
- Pipelined over 4 column chunks of T (512 q-positions each): projections,
  attention, and the output projection of the previous chunk interleave, so
  the PE never drains and the HBM loads/stores overlap compute.
- Softmax exp is split between the Scalar engine (native Exp activation) and
  the Vector engine (Schraudolph bit-trick exp: i32 = s*A + B, bitcast to
  f32 gives 2^(s*log2e*0.125) with ~3% sawtooth error that mostly cancels in
  the softmax normalization). One exp instruction covers a PAIR of score
  tiles ([128, 2, 512] PSUM) to amortize per-instruction overhead.
- Scores are computed transposed (scoresT[kpos, qpos]) so PV needs no
  transposes; the softmax denominator L rides along as a ones-column of V.
- 1/L is computed on a [128, 4] tile (DMA reshape of the one-partition L row)
  instead of a [1, 512] row: the Vector reciprocal is per-lane-serial, so the
  reshape makes it ~100x cheaper.
- Causal trimming: diagonal score tiles only compute/exp/PV the valid
  column range; the triangular boundary block gets a [128,128] mask add.
- RoPE multiplies run on the (otherwise idle) GpSimd engine; V is produced
  via PE transposes of the K/V projection; all PSUM<->SBUF copies are on
  Scalar, masks/normalize/reciprocal on Vector.

All matmuls are float32r (full fp32 data, fast PE mode).
"""

import numpy as np
import sys

sys.path.insert(0, "/opt/trn_rl_repo")

from concourse import bass, bacc, mybir, tile  # noqa: E402
from concourse.bass_utils import run_bass_kernel_spmd  # noqa: E402

F32 = mybir.dt.float32
F32R = mybir.dt.float32r
I16 = mybir.dt.int16
BF16 = mybir.dt.bfloat16

B, T, D = 2, 2048, 1024
HD = 64                      # head dim
NQH = 4                      # query heads per core
QCOLS = NQH * HD             # 256
KC = D // 128                # 8 contraction chunks
NCI = 4                      # 512-wide column chunks of T
NT = T // 128                # 16 k-position tiles
N_CORES = 8

LOG2E = 1.4426950408889634
EXPA = 0.125 * LOG2E * (1 << 7)           # fold the 1/sqrt(hd) scale in
EXPB = (127.0 - 0.05) * (1 << 7)          # Schraudolph bias, tuned offline
MASKV = -300.0

# Scheduling knobs (tuned via the cost-model timeline simulator)
CFG = {
    "rope_engine": "vector",   # "vector" | "gpsimd"
    "lookahead": 1,            # sc pairs emitted ahead of PV
    "pool_mode": "split",      # "shared3" | "split"
    "scp_bufs": 2, "pvp_bufs": 2, "pop_bufs": 2,
    "exp_all_scalar_ci": 1,    # ci <= this -> all exp on Scalar
    "warmup": 32,              # warm-up matmul count
    "op_interleave": True,     # emit outproj tiles between attention heads
    "keepwarm": 0,             # dummy matmuls per head in late chunks
}

_cache = {}


def _r(ap):
    return ap.bitcast(F32R)


def build_nc():
    """Build the (SPMD-identical) single-core bass program."""
    nc = bacc.Bacc("TRN2", target_bir_lowering=False, debug=False)

    xT_d = nc.declare_dram_parameter("xT", [D, T], F32R, isOutput=False)
    wq_d = nc.declare_dram_parameter("wq", [D, QCOLS], F32R, isOutput=False)
    wkv_d = nc.declare_dram_parameter("wkv", [D, 128], F32R, isOutput=False)
    wo_d = nc.declare_dram_parameter("wo", [QCOLS, D], F32R, isOutput=False)
    cos_d = nc.declare_dram_parameter("cosf", [128, T], F32, isOutput=False)
    sin_d = nc.declare_dram_parameter("sinf", [128, T], F32, isOutput=False)
    msk_d = nc.declare_dram_parameter("msk", [128, 128], F32, isOutput=False)
    id_d = nc.declare_dram_parameter("ident", [128, HD], F32, isOutput=False)
    out_d = nc.declare_dram_parameter("out", [T, D], F32, isOutput=True)
    # DRAM scratch used to reshape 1/L rows ([128,4] -> [1,512]); the DMA
    # engine is the only path that can move data across SBUF partitions.
    scr_d = nc.declare_dram_parameter("scr", [16, 512], F32R, isOutput=True)
    ones_d = nc.declare_dram_parameter("ones1", [1, HD], F32R, isOutput=False)

    with tile.TileContext(nc) as tc:
        with (
            tc.tile_pool(name="sb", bufs=1) as sb,
            tc.tile_pool(name="atp", bufs=4) as atp,
            tc.tile_pool(name="aop", bufs=4) as aop,
            tc.tile_pool(name="aotp", bufs=2) as aotp,
            tc.tile_pool(name="otp", bufs=3) as otp,
            tc.tile_pool(name="rotp", bufs=2) as rotp,
            tc.tile_pool(name="vtp", bufs=2) as vtp,
            tc.tile_pool(name="lrp", bufs=2) as lrp,
            tc.tile_pool(name="scp", bufs=CFG["scp_bufs"],
                         space="PSUM") as scp,
            tc.tile_pool(name="pvp", bufs=CFG["pvp_bufs"],
                         space="PSUM") as pvp,
            tc.tile_pool(name="pop", bufs=CFG["pop_bufs"],
                         space="PSUM") as pop,
        ):
            if CFG["pool_mode"] == "shared3":
                pop = scp  # single rotation serves everything but pv
            wq = sb.tile([128, KC, QCOLS], F32, tag="wq")
            wkv = sb.tile([128, KC, 128], F32, tag="wkv")
            wo = sb.tile([128, 2, D], F32, tag="wo")
            cosf = sb.tile([128, T], F32, tag="cosf")
            sinf = sb.tile([128, T], F32, tag="sinf")
            msk = sb.tile([128, 128], F32, tag="msk")
            ident = sb.tile([128, HD], F32, tag="ident")
            qT = [sb.tile([128, T], F32, tag=f"qT{hp}", name=f"qT{hp}")
                  for hp in range(2)]
            kT = sb.tile([128, T], F32, tag="kT")
            vs = sb.tile([128, NT, HD + 1], BF16, tag="vs")
            xc = [sb.tile([128, KC, 512], F32, tag=f"xc{ci}", name=f"xc{ci}")
                  for ci in range(NCI)]

            # --- input loads (order = DMA issue order on the sync queue) ---
            nc.sync.dma_start(
                _r(wq[:]), wq_d[:, :].rearrange("(a b) c -> b a c", a=KC))
            nc.sync.dma_start(
                _r(wkv[:]), wkv_d[:, :].rearrange("(a b) c -> b a c", a=KC))
            for k in range(KC):
                nc.sync.dma_start(
                    _r(xc[0][:, k, :]),
                    xT_d[k * 128:(k + 1) * 128, 0:512])
            nc.sync.dma_start(cosf[:], cos_d[:])
            nc.sync.dma_start(sinf[:], sin_d[:])
            nc.sync.dma_start(msk[:], msk_d[:])
            nc.sync.dma_start(ident[:], id_d[:])
            for k in range(KC):
                nc.sync.dma_start(
                    _r(xc[1][:, k, :]),
                    xT_d[k * 128:(k + 1) * 128, 512:1024])
            nc.sync.dma_start(
                _r(wo[:]), wo_d[:, :].rearrange("(a b) c -> b a c", a=2))
            for ci in range(2, NCI):
                nc.sync.dma_start(
                    _r(xc[ci][:]),
                    xT_d[:, ci * 512:(ci + 1) * 512].rearrange(
                        "(a b) c -> b a c", a=KC))

            ones1 = sb.tile([1, HD], F32, tag="ones1")
            nc.sync.dma_start(_r(ones1[:]), ones_d[:])
            nc.vector.memset(vs[:, :, HD:HD + 1], 1.0)

            def rope_chunk(t_ap, cs, nrows):
                """t = t*cos + rot_half(t)*sin on de-interleaved rows.

                rot DMAs swap 32-row halves of each 64 block; muls/adds run
                on GpSimd to keep Vector/Scalar free for softmax work.
                """
                rot = rotp.tile([128, 512], F32, tag="rot")
                for blk in range(nrows // 64):
                    r0 = blk * 64
                    nc.scalar.dma_start(rot[r0:r0 + 32, :],
                                        t_ap[r0 + 32:r0 + 64, cs])
                    nc.scalar.dma_start(rot[r0 + 32:r0 + 64, :],
                                        t_ap[r0:r0 + 32, cs])
                eng = nc.vector if CFG["rope_engine"] == "vector" else nc.gpsimd
                eng.tensor_mul(_r(t_ap[0:nrows, cs]), t_ap[0:nrows, cs],
                               cosf[0:nrows, cs])
                eng.tensor_mul(rot[0:nrows, :], rot[0:nrows, :],
                               sinf[0:nrows, cs])
                eng.tensor_add(_r(t_ap[0:nrows, cs]), t_ap[0:nrows, cs],
                               rot[0:nrows, :])

            def proj_q(ci, hp):
                cs = slice(ci * 512, (ci + 1) * 512)
                pq = pop.tile([128, 512], F32, tag="sc")
                for k in range(KC):
                    nc.tensor.matmul(
                        pq[:], _r(wq[:, k, hp * 128:(hp + 1) * 128]),
                        _r(xc[ci][:, k, :]),
                        start=(k == 0), stop=(k == KC - 1))
                nc.scalar.copy(_r(qT[hp][:, cs]), pq[:])
                rope_chunk(qT[hp], cs, 128)

            def proj_kv(ci):
                cs = slice(ci * 512, (ci + 1) * 512)
                pkv = pop.tile([128, 512], F32, tag="sc")
                for k in range(KC):
                    nc.tensor.matmul(
                        pkv[:], _r(wkv[:, k, :]), _r(xc[ci][:, k, :]),
                        start=(k == 0), stop=(k == KC - 1))
                nc.scalar.copy(_r(kT[0:64, cs]), pkv[0:64, :])
                vtmp = vtp.tile([128, 512], F32, tag="vtmp")
                nc.scalar.copy(vtmp[64:128, :], pkv[64:128, :])
                rope_chunk(kT, cs, 64)
                # duplicate roped kT into the upper partition half for the
                # odd-head score matmuls (engines can't cross partitions).
                nc.sync.dma_start(_r(kT[64:128, cs]), _r(kT[0:64, cs]))
                # transpose V chunks into natural [tpos, dim] orientation
                for tb in range(4):
                    pt = pop.tile([128, 512], F32, tag="sc")
                    nc.tensor.transpose(
                        pt[:, 0:HD], vtmp[64:128, tb * 128:(tb + 1) * 128],
                        ident[64:128, 0:HD])
                    nc.scalar.copy(vs[:, ci * 4 + tb, 0:HD], pt[:, 0:HD])

            def mk_head(h, ci, pv):
                """Per-head emitters for score pairs and PV accumulation."""
                cs0 = ci * 512
                n_tj = 4 * (ci + 1)
                hp, hr = divmod(h, 2)
                qrow = slice(hr * 64, hr * 64 + 64)

                def sc_emit(p):
                    """Score pair p: 2 matmuls -> exp -> boundary masks."""
                    sc = scp.tile([128, 2, 512], F32, tag="sc")
                    at = atp.tile([128, 2, 512], BF16, tag="at")
                    m0 = 2 * p - 4 * ci
                    wp = 128 * m0 if m0 > 0 else 0
                    for s in range(2):
                        tj = 2 * p + s
                        nc.tensor.matmul(
                            sc[:, s, wp:512],
                            _r(kT[qrow, tj * 128:(tj + 1) * 128]),
                            _r(qT[hp][qrow, cs0 + wp:cs0 + 512]),
                            start=True, stop=True)
                    if ci <= CFG["exp_all_scalar_ci"] or (p + h) % 2 == 0:
                        nc.scalar.activation(
                            at[:, :, wp:512], sc[:, :, wp:512],
                            mybir.ActivationFunctionType.Exp, scale=0.125)
                    else:
                        nc.vector.tensor_scalar(
                            at[:, :, wp:512].bitcast(I16), sc[:, :, wp:512],
                            EXPA, EXPB, mybir.AluOpType.mult,
                            mybir.AluOpType.add)
                    for s in range(2):
                        # causal boundary: zero above-diagonal entries of the
                        # triangular block (0/1 mask; at is SBUF so the
                        # otherwise-idle GpSimd engine can do it).
                        tj = 2 * p + s
                        m = tj - 4 * ci
                        if m >= 0:
                            nc.gpsimd.tensor_mul(
                                at[:, s, 128 * m:128 * m + 128],
                                at[:, s, 128 * m:128 * m + 128], msk[:])
                    return at

                def pv_emit(p, at):
                    for s in range(2):
                        tj = 2 * p + s
                        m = tj - 4 * ci
                        w0 = 128 * m if m > 0 else 0
                        nc.tensor.matmul(
                            pv[:, w0:512], vs[:, tj, :],
                            at[:, s, w0:512],
                            start=(tj == 0), stop=(tj == n_tj - 1),
                            skip_group_check=True)
                return sc_emit, pv_emit

            def attn_headpair(h0, ci, state):
                """Run heads h0 and h0+1 concurrently: their score/PV pair
                streams interleave on the PE, so each head's exp latency
                hides behind the other head's matmuls."""
                n_p = 2 * (ci + 1)
                pvA = pvp.tile([HD + 1, 512], F32, tag="pv", name="pvA")
                pvB = pvp.tile([HD + 1, 512], F32, tag="pv", name="pvB")
                scA, pvAe = mk_head(h0, ci, pvA)
                scB, pvBe = mk_head(h0 + 1, ci, pvB)
                atsA = [scA(0)]
                atsB = [scB(0)]
                for p in range(n_p):
                    if p + 1 < n_p:
                        atsA.append(scA(p + 1))
                        atsB.append(scB(p + 1))
                    pvAe(p, atsA[p])
                    pvBe(p, atsB[p])
                hpA, hrA = divmod(h0, 2)
                hpB, hrB = divmod(h0 + 1, 2)
                state[h0] = (pvA, hpA, hrA, None, None)
                state[h0 + 1] = (pvB, hpB, hrB, None, None)

            def fin_a(h, ci, state):
                # Drain pv: copy L row + unnormalized ao to SBUF, transpose
                # the L row onto partitions ([128,4]) and take 1/L there
                # (engine ops are lane-serial on a 1-partition row, and DMA
                # is the only path that crosses partitions).
                pv, hp, hr, _, _ = state[h]
                lr = lrp.tile([HD + 1, 512], F32, tag="lr")
                nc.vector.tensor_copy(lr[HD:HD + 1, :], pv[HD:HD + 1, :])
                if hr == 0:
                    dst = ao_tiles[ci][hp][0:64, :]
                else:
                    tmp = aotp.tile([64, 512], F32, tag="aotmp", name="aotmp")
                    dst = tmp[:]
                nc.scalar.copy(_r(dst), pv[0:HD, :])  # frees pv for next head
                lcol = pop.tile([128, 512], F32, tag="sc", name="lcol")
                for c in range(4):
                    nc.tensor.transpose(
                        lcol[:, c:c + 1],
                        lr[HD:HD + 1, c * 128:(c + 1) * 128],
                        ident[HD:HD + 1, 0:1])
                linvr = lrp.tile([128, 4], F32, tag="linvr")
                with nc.allow_low_precision(reason="f32r 1/L"):
                    nc.vector.reciprocal(_r(linvr[:]), lcol[:, 0:4])
                state[h] = (pv, hp, hr, dst, linvr)

            def fin_b1(h, ci, state):
                # linvr[p, c] = 1/L[128c + p]: scatter back into qpos order.
                # Emitted one head late so the sync queue never waits on it.
                _, _, _, _, linvr = state[h]
                idx = 4 * ci + h
                nc.sync.dma_start(
                    scr_d[idx:idx + 1, :].rearrange(
                        "a (f p) -> (a p) f", p=128), _r(linvr[:]))

            def fin_b2(h, ci, state):
                # Gather 1/L as a [1,512] row, broadcast across 64 partitions
                # with a ones-stationary matmul, normalize ao in place.
                _, hp, hr, dst, _ = state[h]
                idx = 4 * ci + h
                linv = lrp.tile([1, 512], F32, tag="linv")
                nc.sync.dma_start(_r(linv[:]), scr_d[idx:idx + 1, :])
                lb = pop.tile([128, 512], F32, tag="sc", name="lb")
                nc.tensor.matmul(lb[0:64, :], _r(ones1[:]), _r(linv[:]),
                                 start=True, stop=True)
                nc.vector.tensor_mul(_r(dst), dst, lb[0:64, :])
                if hr == 1:
                    nc.sync.dma_start(_r(ao_tiles[ci][hp][64:128, :]),
                                      _r(dst))

            def keepwarm_burst(n):
                wt = pop.tile([128, 512], F32, tag="sc", name="kw")
                for i in range(n):
                    nc.tensor.matmul(wt[:, 0:QCOLS], _r(wq[:, 0, 0:128]),
                                     _r(wq[:, i % KC, :]),
                                     start=(i == 0), stop=(i == n - 1))

            def attn(ci, ao_ci, parts=()):
                parts = list(parts)
                state = {}
                attn_headpair(0, ci, state)
                fin_a(0, ci, state)
                fin_a(1, ci, state)
                if parts:
                    parts.pop(0)()
                if CFG["keepwarm"] and ci >= 2:
                    keepwarm_burst(CFG["keepwarm"])
                attn_headpair(2, ci, state)
                fin_b1(0, ci, state)
                fin_b1(1, ci, state)
                fin_a(2, ci, state)
                fin_a(3, ci, state)
                fin_b2(0, ci, state)
                fin_b2(1, ci, state)
                for p in parts:
                    p()
                if CFG["keepwarm"] and ci >= 2:
                    keepwarm_burst(CFG["keepwarm"])
                fin_b1(2, ci, state)
                fin_b1(3, ci, state)
                fin_b2(2, ci, state)
                fin_b2(3, ci, state)

            def outproj_tb(ci, tb):
                ao_ci = ao_tiles[ci]
                ta = (ci * 4 + tb) * 128
                ot = otp.tile([128, 2, 512], F32, tag="ot")
                for nh in range(2):
                    po = pop.tile([128, 512], F32, tag="sc")
                    for cc in range(2):
                        nc.tensor.matmul(
                            po[:],
                            _r(ao_ci[cc][:, tb * 128:(tb + 1) * 128]),
                            _r(wo[:, cc, nh * 512:(nh + 1) * 512]),
                            start=(cc == 0), stop=(cc == 1))
                    if nh == 0:
                        nc.scalar.copy(ot[:, nh, :], po[:])
                    else:
                        nc.vector.tensor_copy(ot[:, nh, :], po[:])
                nc.sync.dma_start(out_d[ta:ta + 128, :], ot[:])

            ao_tiles = {}
            for ci in range(NCI):
                ao_tiles[ci] = [
                    aop.tile([128, 512], F32, tag="ao", name=f"ao{ci}_{hp}")
                    for hp in range(2)]

            # Warm-up burst: a dependency-free accumulation run right after
            # the wq load trips the PE HAM clock-gate (4/8 -> 8/8, i.e.
            # 1.2 -> 2.4 GHz needs ~3.4us of sustained busy) before the real
            # work starts.
            nwarm = CFG["warmup"]
            if nwarm:
                warm = pop.tile([128, 512], F32, tag="sc", name="warm")
                for i in range(nwarm):
                    nc.tensor.matmul(warm[:, 0:QCOLS], _r(wq[:, 0, 0:128]),
                                     _r(wq[:, i % KC, :]),
                                     start=(i == 0), stop=(i == nwarm - 1))

            # Emission order: projections run two chunks ahead of attention;
            # out-projection of chunk ci comes after proj(ci+2) so its ao
            # normalization has drained by the time the PE reaches it.
            for c0 in range(2):
                proj_q(c0, 0)
                proj_q(c0, 1)
                proj_kv(c0)
            for ci in range(NCI):
                parts = []
                if CFG["op_interleave"] and ci >= 1:
                    parts = [lambda c=ci - 1, t=tb: outproj_tb(c, t)
                             for tb in range(4)]
                attn(ci, ao_tiles[ci], parts)
                if ci + 2 < NCI:
                    proj_q(ci + 2, 0)
                    proj_q(ci + 2, 1)
                    proj_kv(ci + 2)
                if not CFG["op_interleave"]:
                    for tb in range(4):
                        outproj_tb(ci, tb)
            if CFG["op_interleave"]:
                for tb in range(4):
                    outproj_tb(NCI - 1, tb)

    nc.compile()
    return nc


def _round_f32r(a):
    """Round fp32 to the fp32r grid (11-bit mantissa, round-to-nearest)."""
    bits = np.ascontiguousarray(a, np.float32).view(np.uint32)
    return ((bits + 0x800) & 0xFFFFF000).view(np.float32)


def make_in_maps(x, freqs_cos, freqs_sin, wq, wk, wv, wo):
    """Host-side sharding + layout prep. Returns per-core input dicts."""
    x = np.asarray(x, np.float32)
    fc = np.asarray(freqs_cos, np.float32)
    fs = np.asarray(freqs_sin, np.float32)
    wq = np.asarray(wq, np.float32)
    wk = np.asarray(wk, np.float32)
    wv = np.asarray(wv, np.float32)
    wo = np.asarray(wo, np.float32)

    perm = np.concatenate([np.arange(0, HD, 2), np.arange(1, HD, 2)])
    cosT = np.ascontiguousarray(fc.T)            # (32, T)
    sinT = np.ascontiguousarray(fs.T)
    cosf = np.concatenate([cosT] * 4, axis=0)    # (128, T)
    sinf = np.concatenate([-sinT, sinT, -sinT, sinT], axis=0)

    jj = np.arange(128)[:, None]
    ii = np.arange(128)[None, :]
    msk = np.where(jj <= ii, 1.0, 0.0).astype(np.float32)
    ident = np.tile(np.eye(HD, dtype=np.float32), (2, 1))

    in_maps = []
    for c in range(N_CORES):
        b, g = divmod(c, 4)
        wq_c = wq[:, g * QCOLS:(g + 1) * QCOLS]
        wq_c = np.ascontiguousarray(
            wq_c.reshape(D, NQH, HD)[:, :, perm].reshape(D, QCOLS))
        wk_c = wk[:, g * HD:(g + 1) * HD][:, perm]
        wv_c = wv[:, g * HD:(g + 1) * HD]
        wkv_c = np.ascontiguousarray(np.concatenate([wk_c, wv_c], axis=1))
        wo_c = np.ascontiguousarray(wo[g * QCOLS:(g + 1) * QCOLS, :])
        xT_c = np.ascontiguousarray(x[b].T)
        in_maps.append({
            "xT": _round_f32r(xT_c), "wq": _round_f32r(wq_c),
            "wkv": _round_f32r(wkv_c), "wo": _round_f32r(wo_c),
            "cosf": cosf, "sinf": sinf, "msk": msk, "ident": ident,
            "ones1": np.ones((1, HD), np.float32),
        })
    return in_maps


def run_on_cores(in_maps, trace=False, **kwargs):
    if "nc" not in _cache:
        _cache["nc"] = build_nc()
    return run_bass_kernel_spmd(
        _cache["nc"], in_maps, core_ids=list(range(N_CORES)), trace=trace,
        **kwargs)


def kernel(x, freqs_cos, freqs_sin, wq, wk, wv, wo):
    in_maps = make_in_maps(x, freqs_cos, freqs_sin, wq, wk, wv, wo)
    res = run_on_cores(in_maps)
    outs = [res.results[c]["out"] for c in range(N_CORES)]
    full = np.empty((B, T, D), np.float32)
    for b in range(B):
        full[b] = outs[4 * b] + outs[4 * b + 1] + outs[4 * b + 2] + outs[4 * b + 3]
    return full
